# revision 27
# baseline (speedup 1.0000x reference)
"""AgentAttention Trainium2 kernel.

Full inputs -> shard batch over 8 NeuronCores (2 samples each) -> Bass/Tile
kernel per core -> gather full output.

Per-sample pipeline (feature-major X=[256,4096] is the natural DRAM layout):
  QKV projections (bf16 matmuls); agent tokens via pooled-X (pooling commutes
  with Wq); token-major scores [n_tile, (head,agent)=392] from block-diagonal
  agent rhs; softmax via on-device-built exp(bias) tables; stage-1 colsum and
  agent_v accumulated in PSUM across the 32-tile sweep; stage-2 probs
  PE-transposed for the fused (Wproj @ AV) output matmul; depthwise 3x3 conv
  as PE diagonal matmuls on a zero-padded 66x66 V image, folded into the
  final projection with bproj_eff = bproj + Wproj @ bdwc.
"""
import sys
import numpy as np
from contextlib import ExitStack

sys.path.insert(0, "/opt/trn_rl_repo")

import concourse.bass as bass
import concourse.tile as tile
from concourse import mybir
from concourse.bass_utils import run_bass_kernel_spmd

DEBUG_OUT = False
B, DIM, HEADS, AGENT = 16, 256, 8, 49
H = W = 64
N = H * W
HD = DIM // HEADS             # 32
SCALE = HD ** -0.5
N_CORES = 8
SPC = B // N_CORES            # 2 samples per core
NT = N // 128                 # 32 token tiles
HA = HEADS * AGENT            # 392
PAIRW = 2 * AGENT             # 98
NPAIR = 4
KB = 49 + 64 + 64             # 177 basis terms
PADW = 66

F32 = mybir.dt.float32
BF16 = mybir.dt.float16  # fp16: same PE/DVE speed as bf16, 8x finer mantissa
AX = mybir.AxisListType
AF = mybir.ActivationFunctionType


def _pool_bins(size, out):
    return [((i * size) // out, -((-(i + 1) * size) // out)) for i in range(out)]


def _resize_matrix(n_in, n_out):
    """Row-normalized half-pixel triangle-kernel resize matrix: matches
    jax.image.resize(method='bilinear') upsampling."""
    R = np.zeros((n_out, n_in), np.float64)
    scale = n_in / n_out
    for o in range(n_out):
        c = (o + 0.5) * scale - 0.5
        for i in range(n_in):
            R[o, i] = max(0.0, 1.0 - abs(c - i))
        s = R[o].sum()
        if s > 0:
            R[o] /= s
    return R.astype(np.float32)


def _wt_layout(WT):
    """[256, 256] (cin, cout) -> [128, 2, 256] tile layout, [p, k, m]."""
    return np.ascontiguousarray(WT.reshape(2, 128, DIM).transpose(1, 0, 2))


def build_host_constants(Wq, Wkv, Wproj, bproj, Wdwc, bdwc,
                         an_bias, na_bias, ah_bias, aw_bias, ha_bias, wa_bias):
    """Pure parameter re-layout / folding on host."""
    c = {}
    Wk, Wv = Wkv[:DIM], Wkv[DIM:]
    c["WqT"] = _wt_layout(Wq.T)
    c["WkT"] = _wt_layout(Wk.T)
    c["WvT"] = _wt_layout(Wv.T)
    c["WprojT"] = _wt_layout(Wproj.T)

    Rh = _resize_matrix(7, H)
    Rw = _resize_matrix(7, W)
    Phi = np.zeros((N, KB), np.float32)
    for y in range(H):
        for x in range(W):
            n = y * W + x
            Phi[n, :49] = np.outer(Rh[y], Rw[x]).reshape(49)
            Phi[n, 49 + y] = 1.0
            Phi[n, 113 + x] = 1.0
    PhiT = np.ascontiguousarray(Phi.T)
    c["PhiA"] = PhiT[:128]
    c["PhiB"] = PhiT[128:]

    C1 = np.zeros((KB, HA), np.float32)
    C2 = np.zeros((KB, HA), np.float32)
    for h in range(HEADS):
        for a in range(AGENT):
            col = h * AGENT + a
            C1[:49, col] = an_bias[h, a].reshape(49)
            C1[49:113, col] = ah_bias[0, h, a, :, 0]
            C1[113:, col] = aw_bias[0, h, a, 0, :]
            C2[:49, col] = na_bias[h, a].reshape(49)
            C2[49:113, col] = ha_bias[0, h, :, 0, a]
            C2[113:, col] = wa_bias[0, h, 0, :, a]
    Cfull = np.concatenate([C1, C2], axis=1)
    c["CA"] = np.ascontiguousarray(Cfull[:128])
    c["CB"] = np.ascontiguousarray(Cfull[128:])

    binsH, binsW = _pool_bins(H, 7), _pool_bins(W, 7)
    s49 = np.zeros((1, AGENT), np.float32)
    for p in range(7):
        for q in range(7):
            lp = binsH[p][1] - binsH[p][0]
            lq = binsW[q][1] - binsW[q][0]
            s49[0, p * 7 + q] = 1.0 / (lp * lq)
    c["S49"] = s49

    w9 = Wdwc[:, 0, :, :].reshape(DIM, 9)
    diag = np.zeros((18, 128, 128), np.float32)
    for t in range(9):
        for ct_ in range(2):
            np.fill_diagonal(diag[t * 2 + ct_],
                             w9[ct_ * 128:(ct_ + 1) * 128, t])
    c["DIAG"] = diag.reshape(18 * 128, 128)

    c["bproj_eff"] = (bproj + Wproj @ bdwc).astype(np.float32).reshape(1, 256)
    c["onesN"] = np.ones((1, 512), np.float32)
    c["ident1"] = np.eye(1, dtype=np.float32)
    c["zeroR"] = np.zeros((1, 128), np.float32)
    c["ident_bf"] = np.eye(128, dtype=np.float32)
    c["ones1"] = np.ones((128, 1), np.float32)
    return c


CONST_SPECS = [
    ("WqT", BF16, [128, 2, DIM]),
    ("WkT", BF16, [128, 2, DIM]),
    ("WvT", BF16, [128, 2, DIM]),
    ("WprojT", BF16, [128, 2, DIM]),
    ("PhiA", BF16, [128, N]),
    ("PhiB", BF16, [49, N]),
    ("CA", BF16, [128, 2 * HA]),
    ("CB", BF16, [49, 2 * HA]),
    ("S49", F32, [1, AGENT]),
    ("bproj_eff", BF16, [1, DIM]),
    ("onesN", BF16, [1, 512]),
    ("ident1", F32, [1, 1]),
    ("zeroR", BF16, [1, 128]),
    ("ident_bf", BF16, [128, 128]),
    ("ones1", BF16, [128, 1]),
]
DIAG_SPEC = ("DIAG", BF16, [18 * 128, 128])   # DRAM-resident, streamed


def split_multiwaits(nc, max_waits=1):
    """Walrus codegen has one sync-wait slot per instruction; split extras
    into standalone EventSemaphore waits on the same engine."""
    n_split = 0
    for f in nc.m.functions:
        for bb in f.blocks:
            new_insts = []
            changed = False
            for inst in bb.instructions:
                si = inst.sync_info
                if (si is not None and si.on_wait is not None
                        and len(si.on_wait) > max_waits and inst.is_executable()):
                    waits = list(si.on_wait)
                    extra, keep = waits[:-max_waits], waits[-max_waits:]
                    for w in extra:
                        ev = mybir.InstEventSemaphore(
                            name=f"{inst.name}-ws{n_split}",
                            engine=inst.engine, ins=[], outs=[],
                            sync_info=mybir.SyncInfo(on_wait=[w], on_update=[]),
                        )
                        new_insts.append(ev)
                        n_split += 1
                    inst.sync_info = mybir.SyncInfo(
                        on_wait=keep, on_update=list(si.on_update))
                    changed = True
                new_insts.append(inst)
            if changed:
                bb.instructions = new_insts
    return n_split


def build_nc():
    nc = bass.Bass()
    x_in = nc.dram_tensor("x", [SPC, DIM, N], BF16, kind="ExternalInput")
    out_d = nc.dram_tensor("out", [SPC, DIM, N], F32, kind="ExternalOutput")
    dbg = None if not DEBUG_OUT else {
        "agentT": nc.dram_tensor("dbg_agentT", [SPC, DIM, AGENT], F32,
                                 kind="ExternalOutput"),
        "cs": nc.dram_tensor("dbg_cs", [SPC, 1, HA], F32, kind="ExternalOutput"),
        "avbd": nc.dram_tensor("dbg_avbd", [SPC, DIM, 196], F32,
                               kind="ExternalOutput"),
        "dwout": nc.dram_tensor("dbg_dwout", [SPC, DIM, N], F32,
                                kind="ExternalOutput"),
    }
    cst = {name: nc.dram_tensor(name, shape, dt, kind="ExternalInput")
           for name, dt, shape in CONST_SPECS}
    name, dt, shape = DIAG_SPEC
    cst[name] = nc.dram_tensor(name, shape, dt, kind="ExternalInput")

    with tile.TileContext(nc) as tc, ExitStack() as ctx:
        kernel_body(ctx, tc, nc, x_in, out_d, cst, dbg)
    split_multiwaits(nc)
    return nc


def kernel_body(ctx, tc, nc, x_in, out_d, cst, dbg=None):
    const = ctx.enter_context(tc.tile_pool(name="const", bufs=1))
    big = ctx.enter_context(tc.tile_pool(name="big", bufs=1))
    work = ctx.enter_context(tc.tile_pool(name="work", bufs=3))
    small = ctx.enter_context(tc.tile_pool(name="small", bufs=2))
    outp = ctx.enter_context(tc.tile_pool(name="outp", bufs=2))
    # PSUM budget (8 banks): psA 2 + psB 2 + psAcc 3
    psA = ctx.enter_context(tc.tile_pool(name="psA", bufs=3, space="PSUM"))
    psB = ctx.enter_context(tc.tile_pool(name="psB", bufs=2, space="PSUM"))
    psAcc = ctx.enter_context(tc.tile_pool(name="psAcc", bufs=1, space="PSUM"))

    ct = {}
    for name, dt, shape in CONST_SPECS:
        t = const.tile(shape, dt, name=f"c_{name}")
        nc.sync.dma_start(out=t, in_=cst[name].ap())
        ct[name] = t
    s49rep = const.tile([128, AGENT], F32, name="s49rep")
    nc.sync.dma_start(
        out=s49rep,
        in_=bass.AP(tensor=cst["S49"], offset=0, ap=[[0, 128], [1, AGENT]]))
    ct["s49rep"] = s49rep
    ct["DIAG_dram"] = cst["DIAG"]

    for s in range(SPC):
        sample(nc, ct, s, x_in, out_d, big, work, small, outp,
               psA, psB, psAcc, dbg)


def sample(nc, ct, s, x_in, out_d, big, work, small, outp,
           psA, psB, psAcc, dbg=None):
    F = 512
    NCH = N // F

    # ---- X halves (bf16) ------------------------------------------------
    xh = []
    for hf in range(2):
        t = big.tile([128, N], BF16, name=f"x{hf}", tag=f"xh{hf}")
        nc.sync.dma_start(out=t, in_=x_in.ap()[s, hf * 128:(hf + 1) * 128, :])
        xh.append(t)

    # ---- QKV projections ------------------------------------------------
    QT, KT, VT = [], [], []
    for hf in range(2):
        QT.append(big.tile([128, N], BF16, name=f"qt{hf}", tag=f"qt{hf}"))
        KT.append(big.tile([128, N], BF16, name=f"kt{hf}", tag=f"kt{hf}"))
        VT.append(big.tile([128, N], BF16, name=f"v{hf}", tag=f"v{hf}"))

    for wname, dest in (("WvT", "v"), ("WkT", "k"), ("WqT", "q")):
        wt = ct[wname]
        for mt in range(2):
            for chn in range(NCH):
                ps = psA.tile([128, F], F32, name="ps_proj", tag="ps_main")
                for kt_ in range(2):
                    nc.tensor.matmul(
                        ps,
                        lhsT=wt[:, kt_, mt * 128:(mt + 1) * 128],
                        rhs=xh[kt_][:, chn * F:(chn + 1) * F],
                        start=(kt_ == 0), stop=(kt_ == 1))
                if dest == "q":
                    nc.vector.tensor_copy(out=QT[mt][:, chn * F:(chn + 1) * F],
                                          in_=ps)
                elif dest == "k":
                    nc.vector.tensor_copy(out=KT[mt][:, chn * F:(chn + 1) * F],
                                          in_=ps)
                else:
                    nc.scalar.copy(out=VT[mt][:, chn * F:(chn + 1) * F], in_=ps)

    # ---- agent tokens ---------------------------------------------------
    binsH, binsW = _pool_bins(H, 7), _pool_bins(W, 7)
    XpH = []
    for hf in range(2):
        x3 = xh[hf].rearrange("p (y x) -> p y x", y=H)
        qx = small.tile([128, H, 7], F32, name="qx", tag="qx")
        for q, (s0, e0) in enumerate(binsW):
            nc.vector.tensor_reduce(
                out=qx[:, :, q:q + 1], in_=x3[:, :, s0:e0],
                axis=AX.X, op=mybir.AluOpType.add)
        xp = small.tile([128, 7, 7], F32, name="xp", tag="xp")
        qxf = qx.rearrange("p y q -> p (y q)")
        for p, (s0, e0) in enumerate(binsH):
            seg = bass.AP(tensor=qxf.tensor, offset=qxf.offset + s0 * 7,
                          ap=[qxf.ap[0], [1, 7], [7, e0 - s0]])
            nc.vector.tensor_reduce(
                out=xp[:, p, :], in_=seg, axis=AX.X, op=mybir.AluOpType.add)
        xpb = small.tile([128, AGENT], BF16, name="xpb", tag="xpb")
        nc.vector.tensor_mul(
            out=xpb, in0=xp.rearrange("p a b -> p (a b)"), in1=ct["s49rep"])
        XpH.append(xpb)

    agentT = []
    for mt in range(2):
        ps = psB.tile([128, AGENT], F32, name="ps_ag", tag="ps_aux")
        for kt_ in range(2):
            nc.tensor.matmul(
                ps,
                lhsT=ct["WqT"][:, kt_, mt * 128:(mt + 1) * 128],
                rhs=XpH[kt_], start=(kt_ == 0), stop=(kt_ == 1))
        at = small.tile([128, AGENT], BF16, name=f"at{mt}", tag=f"at{mt}")
        nc.scalar.activation(out=at, in_=ps, func=AF.Copy, scale=SCALE)
        agentT.append(at)
        if dbg is not None:
            atd = small.tile([128, AGENT], F32, name="atd", tag="atd")
            nc.vector.tensor_copy(out=atd, in_=at)
            nc.sync.dma_start(
                out=dbg["agentT"].ap()[s, mt * 128:(mt + 1) * 128, :], in_=atd)

    bd = []
    for hf in range(2):
        b = small.tile([128, 256], BF16, name=f"bd{hf}", tag=f"bd{hf}")
        nc.vector.memset(b, 0.0)
        for hl in range(4):
            nc.vector.tensor_copy(
                out=b[hl * 32:(hl + 1) * 32, hl * AGENT:(hl + 1) * AGENT],
                in_=agentT[hf][hl * 32:(hl + 1) * 32, :])
        bd.append(b)

    # ---- dwc: 3x3 depthwise conv as PE diagonal matmuls -----------------
    # Flat shifts delta=dy*64+dx on plain V; y-edge reads clip naturally
    # (zero-pad semantics); x-wrap columns (x=0,63) recomputed exactly with
    # strided correction matmuls. DW shares the xh1 slot (X dead by now).
    DWall = big.tile([128, 2, N], BF16, name="dwall", tag="xh1")
    TAPS = [(0, 0)] + [(dy, dx) for dy in (-1, 0, 1) for dx in (-1, 0, 1)
                       if (dy, dx) != (0, 0)]
    for cti in range(2):
        dgs = work.tile([128, 9, 128], BF16, name="dgs", tag="dgs")
        nc.sync.dma_start(
            out=dgs,
            in_=bass.AP(tensor=ct["DIAG_dram"], offset=cti * 128 * 128,
                        ap=[[128, 128], [2 * 128 * 128, 9], [1, 128]]))
        v = VT[cti]
        for chn in range(NCH):
            ps = psA.tile([128, F], F32, name="ps_dw", tag="ps_dw", bufs=1)
            lo = chn * F
            for k, (dy, dx) in enumerate(TAPS):
                t9 = (dy + 1) * 3 + (dx + 1)
                d = dy * W + dx
                a = max(0, -(lo + d))
                b_ = max(0, (lo + F + d) - N)
                nc.tensor.matmul(
                    ps[:, a:F - b_], lhsT=dgs[:, t9, :],
                    rhs=v[:, lo + d + a:lo + F + d - b_],
                    start=(k == 0), stop=False, skip_group_check=True)
            # x-boundary corrections: recompute x=0 and x=63 columns
            r0 = chn * 8
            for xb, dxs in ((0, (0, 1)), (W - 1, (-1, 0))):
                first = True
                for dy in (-1, 0, 1):
                    for dx in dxs:
                        t9 = (dy + 1) * 3 + (dx + 1)
                        rs = max(r0, -dy)
                        re = min(r0 + 8, H - max(0, dy))
                        nr = re - rs
                        out_ap = bass.AP(
                            tensor=ps.tensor,
                            offset=ps.offset + (rs - r0) * W + xb,
                            ap=[ps.ap[0], [W, nr]])
                        rhs_ap = bass.AP(
                            tensor=v.tensor,
                            offset=v.offset + (rs + dy) * W + xb + dx,
                            ap=[v.ap[0], [W, nr]])
                        nc.tensor.matmul(
                            out_ap, lhsT=dgs[:, t9, :], rhs=rhs_ap,
                            start=first, stop=False, skip_group_check=True)
                        first = False
            nc.scalar.copy(out=DWall[:, cti, lo:lo + F], in_=ps)

    # ---- stage 1: agent -> kv attention (with inline V transpose) -------
    # ps_av[g] packs 4 heads: head h -> tile h//4, rows 64*(h%2),
    # cols 32*((h//2)%2) -- aligned partition bases for later slicing.
    ps_cs = psAcc.tile([1, HA], F32, name="ps_cs", tag="ps_cs")
    ps_av8 = psAcc.tile([128, 128], F32, name="ps_av8", tag="ps_av8")
    ps_av = [ps_av8[:, :64], ps_av8[:, 64:]]
    # One full-tile start=True clear: per-group start flags would wipe
    # sibling groups' has_written bits in the shared bank.
    nc.tensor.matmul(ps_av8, lhsT=ct["zeroR"], rhs=ct["onesN"][:, :128],
                     start=True, stop=False, skip_group_check=True)
    for t in range(NT):
        # V token-major tile [128, 256] via PE transpose of V slices
        vtok = work.tile([128, DIM], BF16, name="vtok", tag="vtok")
        for hf in range(2):
            pst = psB.tile([128, 128], BF16, name="ps_vt", tag="ps_aux")
            nc.tensor.transpose(
                pst, in_=VT[hf][:, t * 128:(t + 1) * 128],
                identity=ct["ident_bf"])
            nc.vector.tensor_copy(out=vtok[:, hf * 128:(hf + 1) * 128],
                                  in_=pst)

        ps = psA.tile([128, 512], F32, name="ps_s1", tag="ps_main")
        nc.tensor.matmul(
            ps[:, :HA], lhsT=ct["PhiA"][:, t * 128:(t + 1) * 128],
            rhs=ct["CA"][:, :HA], start=True, stop=False,
            skip_group_check=True)
        nc.tensor.matmul(
            ps[:, :HA], lhsT=ct["PhiB"][:, t * 128:(t + 1) * 128],
            rhs=ct["CB"][:, :HA], start=False, stop=False,
            skip_group_check=True)
        for hf in range(2):
            nc.tensor.matmul(
                ps[:, hf * 196:(hf + 1) * 196],
                lhsT=KT[hf][:, t * 128:(t + 1) * 128],
                rhs=bd[hf][:, :196], start=False, stop=True,
                skip_group_check=True)
        e1 = work.tile([128, HA], BF16, name="e1", tag="e")
        nc.scalar.activation(out=e1, in_=ps[:, :HA], func=AF.Exp)
        nc.tensor.matmul(ps_cs, lhsT=ct["ones1"], rhs=e1,
                         start=(t == 0), stop=(t == NT - 1),
                         skip_group_check=True)
        for h in range(HEADS):
            rb, cb = 64 * (h % 2), 32 * ((h // 2) % 2)
            nc.tensor.matmul(
                ps_av[h // 4][rb:rb + AGENT, cb:cb + HD],
                lhsT=e1[:, h * AGENT:(h + 1) * AGENT],
                rhs=vtok[:, h * HD:(h + 1) * HD],
                start=False, stop=(t == NT - 1),
                skip_group_check=True)

    # stage-1 normalize -> AVbd -> WpAVT
    cs_sb = small.tile([1, HA], F32, name="cs_sb", tag="cs_sb")
    nc.scalar.copy(out=cs_sb, in_=ps_cs)
    if dbg is not None:
        nc.sync.dma_start(out=dbg["cs"].ap()[s], in_=cs_sb)
    AVbd = []
    for hf in range(2):
        av = small.tile([128, 196], BF16, name=f"avbd{hf}", tag=f"avbd{hf}")
        nc.vector.memset(av, 0.0)
        AVbd.append(av)
    for h in range(HEADS):
        rb, cb = 64 * (h % 2), 32 * ((h // 2) % 2)
        pst = psB.tile([AGENT, 1], F32, name="ps_csT", tag="ps_aux")
        nc.tensor.transpose(
            pst, in_=cs_sb[:, h * AGENT:(h + 1) * AGENT],
            identity=ct["ident1"])
        rcp = small.tile([AGENT, 1], F32, name="rcp", tag="rcp")
        nc.vector.reciprocal(out=rcp, in_=pst)
        avn = small.tile([AGENT, HD], BF16, name="avn", tag="avn")
        nc.vector.tensor_scalar_mul(
            out=avn, in0=ps_av[h // 4][rb:rb + AGENT, cb:cb + HD],
            scalar1=rcp)
        pst2 = psB.tile([HD, AGENT], BF16, name="ps_avT", tag="ps_aux")
        nc.tensor.transpose(
            pst2, in_=avn, identity=ct["ident_bf"][:AGENT, :AGENT])
        hf, hl = h // 4, h % 4
        nc.scalar.copy(
            out=AVbd[hf][hl * HD:(hl + 1) * HD,
                         hl * AGENT:(hl + 1) * AGENT],
            in_=pst2)

    if dbg is not None:
        for hf in range(2):
            avd = small.tile([128, 196], F32, name="avd", tag="avd")
            nc.vector.tensor_copy(out=avd, in_=AVbd[hf])
            nc.sync.dma_start(
                out=dbg["avbd"].ap()[s, hf * 128:(hf + 1) * 128, :], in_=avd)
    WpAVT = []
    for p in range(NPAIR):
        hf = p // 2
        ps = psB.tile([PAIRW, DIM], F32, name="ps_wpav", tag="ps_aux")
        nc.tensor.matmul(
            ps, lhsT=AVbd[hf][:, (p % 2) * PAIRW:(p % 2) * PAIRW + PAIRW],
            rhs=ct["WprojT"][:, hf, :],
            start=True, stop=True)
        w = small.tile([PAIRW, DIM], BF16, name=f"wpav{p}", tag=f"wpav{p}")
        nc.scalar.copy(out=w, in_=ps)
        WpAVT.append(w)

    # ---- stage 2 (A2T shares the kt0 slot; K dead after stage 1) -------
    A2Tall = big.tile([PAIRW, NPAIR, N], BF16, name="a2tall", tag="kt0")
    for t in range(NT):
        ps = psA.tile([128, 512], F32, name="ps_s2", tag="ps_main")
        nc.tensor.matmul(
            ps[:, :HA], lhsT=ct["PhiA"][:, t * 128:(t + 1) * 128],
            rhs=ct["CA"][:, HA:], start=True, stop=False,
            skip_group_check=True)
        nc.tensor.matmul(
            ps[:, :HA], lhsT=ct["PhiB"][:, t * 128:(t + 1) * 128],
            rhs=ct["CB"][:, HA:], start=False, stop=False,
            skip_group_check=True)
        for hf in range(2):
            nc.tensor.matmul(
                ps[:, hf * 196:(hf + 1) * 196],
                lhsT=QT[hf][:, t * 128:(t + 1) * 128],
                rhs=bd[hf][:, :196], start=False, stop=True,
                skip_group_check=True)
        e2 = work.tile([128, HA], BF16, name="e2", tag="e")
        nc.scalar.activation(out=e2, in_=ps[:, :HA], func=AF.Exp)
        s2 = work.tile([128, HEADS], F32, name="s2", tag="s2")
        nc.vector.tensor_reduce(
            out=s2, in_=e2.rearrange("p (h a) -> p h a", h=HEADS),
            axis=AX.X, op=mybir.AluOpType.add)
        r2 = work.tile([128, HEADS], F32, name="r2", tag="r2")
        nc.vector.reciprocal(out=r2, in_=s2)
        a2 = work.tile([128, HA], BF16, name="a2", tag="a2")
        r2v = bass.AP(tensor=r2.tensor, offset=r2.offset,
                      ap=[r2.ap[0], [1, HEADS], [0, AGENT]])
        nc.vector.tensor_mul(
            out=a2.rearrange("p (h a) -> p h a", h=HEADS),
            in0=e2.rearrange("p (h a) -> p h a", h=HEADS), in1=r2v)
        for p in range(NPAIR):
            pst = psB.tile([PAIRW, 128], BF16, name="ps_a2t", tag="ps_aux")
            nc.tensor.transpose(
                pst, in_=a2[:, p * PAIRW:(p + 1) * PAIRW],
                identity=ct["ident_bf"])
            eng = nc.vector if p % 2 == 0 else nc.scalar
            eng.tensor_copy(out=A2Tall[:, p, t * 128:(t + 1) * 128], in_=pst) \
                if p % 2 == 0 else nc.scalar.copy(
                    out=A2Tall[:, p, t * 128:(t + 1) * 128], in_=pst)

    # ---- fused output ---------------------------------------------------
    for ot in range(2):
        for chn in range(NCH):
            ps = psA.tile([128, F], F32, name="ps_o", tag="ps_main")
            for p in range(NPAIR):
                nc.tensor.matmul(
                    ps, lhsT=WpAVT[p][:, ot * 128:(ot + 1) * 128],
                    rhs=A2Tall[:, p, chn * F:(chn + 1) * F],
                    start=(p == 0), stop=False)
            for kt_ in range(2):
                nc.tensor.matmul(
                    ps,
                    lhsT=ct["WprojT"][:, kt_, ot * 128:(ot + 1) * 128],
                    rhs=DWall[:, kt_, chn * F:(chn + 1) * F],
                    start=False, stop=False)
            nc.tensor.matmul(
                ps, lhsT=ct["bproj_eff"][:, ot * 128:(ot + 1) * 128],
                rhs=ct["onesN"], start=False, stop=True)
            o = outp.tile([128, F], F32, name="o_st", tag="o_st")
            nc.scalar.copy(out=o, in_=ps)
            nc.gpsimd.dma_start(
                out=out_d.ap()[s, ot * 128:(ot + 1) * 128,
                               chn * F:(chn + 1) * F],
                in_=o)
            if dbg is not None:
                ps2 = psA.tile([128, F], F32, name="ps_dbg", tag="ps_main")
                for kt_ in range(2):
                    nc.tensor.matmul(
                        ps2,
                        lhsT=ct["WprojT"][:, kt_, ot * 128:(ot + 1) * 128],
                        rhs=DWall[:, kt_, chn * F:(chn + 1) * F],
                        start=(kt_ == 0), stop=(kt_ == 1))
                o2 = outp.tile([128, F], F32, name="o_dbg", tag="o_st")
                nc.scalar.copy(out=o2, in_=ps2)
                nc.sync.dma_start(
                    out=dbg["dwout"].ap()[s, ot * 128:(ot + 1) * 128,
                                          chn * F:(chn + 1) * F],
                    in_=o2)


def kernel(**inputs):
    import ml_dtypes
    x = np.asarray(inputs["x"], np.float32)
    host = build_host_constants(
        np.asarray(inputs["Wq"], np.float32),
        np.asarray(inputs["Wkv"], np.float32),
        np.asarray(inputs["Wproj"], np.float32),
        np.asarray(inputs["bproj"], np.float32),
        np.asarray(inputs["Wdwc"], np.float32),
        np.asarray(inputs["bdwc"], np.float32),
        np.asarray(inputs["an_bias"], np.float32),
        np.asarray(inputs["na_bias"], np.float32),
        np.asarray(inputs["ah_bias"], np.float32),
        np.asarray(inputs["aw_bias"], np.float32),
        np.asarray(inputs["ha_bias"], np.float32),
        np.asarray(inputs["wa_bias"], np.float32),
    )
    nc = build_nc()

    specs = CONST_SPECS + [DIAG_SPEC]
    dts = {name: dt for name, dt, _ in specs}

    def cast(name, arr):
        if dts[name] == BF16:
            return np.asarray(arr, np.float16)
        return np.asarray(arr, np.float32)

    const_map = {name: cast(name, host[name]) for name, _, _ in specs}
    xs = x.reshape(B, DIM, N)
    in_maps = []
    for c in range(N_CORES):
        m = dict(const_map)
        m["x"] = np.asarray(xs[c * SPC:(c + 1) * SPC], np.float16)
        in_maps.append(m)

    res = run_bass_kernel_spmd(nc, in_maps, core_ids=list(range(N_CORES)))
    out = np.concatenate([res.results[c]["out"] for c in range(N_CORES)],
                         axis=0)
    return out.reshape(B, DIM, H, W)


# revision 29
# speedup vs baseline: 1.0224x; 1.0224x over previous
"""AgentAttention Trainium2 kernel.

Full inputs -> shard batch over 8 NeuronCores (2 samples each) -> Bass/Tile
kernel per core -> gather full output.

Layout: feature-major X=[256,4096]; token-major scores [n_tile, head*64]
with the agent dim padded 49->64 per head (pad bias = -30 so exp()==0),
keeping every PE operand 32/64-aligned. Position biases are rebuilt per
tile into the score PSUM via a 177-term basis matmul (kron(Rh,Rw) resize +
y/x one-hots). Stage-1 colsum + agent_v accumulate in PSUM across the
32-tile sweep; stage-2 probs are PE-transposed for the fused
(Wproj @ agent_v) output matmul. The 3x3 depthwise conv runs as PE
diagonal matmuls with flat shifts (y edges clip = zero-pad); the x-wrap
columns are recomputed exactly by DVE strided ops. bproj_eff folds
Wproj @ bdwc.
"""
import sys
import numpy as np
from contextlib import ExitStack

sys.path.insert(0, "/opt/trn_rl_repo")

import concourse.bass as bass
import concourse.tile as tile
from concourse import mybir
from concourse.bass_utils import run_bass_kernel_spmd

DEBUG_OUT = False
B, DIM, HEADS, AGENT = 16, 256, 8, 49
H = W = 64
N = H * W
HD = DIM // HEADS             # 32
SCALE = HD ** -0.5
N_CORES = 8
SPC = B // N_CORES            # 2 samples per core
NT = N // 128                 # 32 token tiles
AGP = 64                      # padded agent dim per head
HAP = HEADS * AGP             # 512
KB = 49 + 64 + 64             # 177 basis terms
NPAIR = 4

F32 = mybir.dt.float32
F16 = mybir.dt.float16
AX = mybir.AxisListType
AF = mybir.ActivationFunctionType


def _pool_bins(size, out):
    return [((i * size) // out, -((-(i + 1) * size) // out)) for i in range(out)]


def _resize_matrix(n_in, n_out):
    """Row-normalized half-pixel triangle-kernel resize matrix: matches
    jax.image.resize(method='bilinear') upsampling."""
    R = np.zeros((n_out, n_in), np.float64)
    scale = n_in / n_out
    for o in range(n_out):
        c = (o + 0.5) * scale - 0.5
        for i in range(n_in):
            R[o, i] = max(0.0, 1.0 - abs(c - i))
        s = R[o].sum()
        if s > 0:
            R[o] /= s
    return R.astype(np.float32)


def _wt_layout(WT):
    """[256, 256] (cin, cout) -> [128, 2, 256] tile layout, [p, k, m]."""
    return np.ascontiguousarray(WT.reshape(2, 128, DIM).transpose(1, 0, 2))


def build_host_constants(Wq, Wkv, Wproj, bproj, Wdwc, bdwc,
                         an_bias, na_bias, ah_bias, aw_bias, ha_bias, wa_bias):
    """Pure parameter re-layout / folding on host."""
    c = {}
    Wk, Wv = Wkv[:DIM], Wkv[DIM:]
    c["WqT"] = _wt_layout(Wq.T)
    c["WkT"] = _wt_layout(Wk.T)
    c["WvT"] = _wt_layout(Wv.T)
    c["WprojT"] = _wt_layout(Wproj.T)

    Rh = _resize_matrix(7, H)
    Rw = _resize_matrix(7, W)
    Phi = np.zeros((N, KB), np.float32)
    for y in range(H):
        for x in range(W):
            n = y * W + x
            Phi[n, :49] = np.outer(Rh[y], Rw[x]).reshape(49)
            Phi[n, 49 + y] = 1.0
            Phi[n, 113 + x] = 1.0
    PhiT = np.ascontiguousarray(Phi.T)
    c["PhiA"] = PhiT[:128]
    c["PhiB"] = PhiT[128:]

    # C tables [177, 2*HAP] in 64-padded (h, a) column order. Pad columns
    # get -10 in each of the three basis groups (kron rows weight to 1,
    # one-hot groups contribute 1 each) -> bias -30 -> exp == 0 in fp16.
    Cfull = np.zeros((KB, 2 * HAP), np.float32)
    for g in range(2):
        for h in range(HEADS):
            for a in range(AGP):
                col = g * HAP + h * AGP + a
                if a >= AGENT:
                    Cfull[:, col] = -10.0
                    continue
                if g == 0:
                    Cfull[:49, col] = an_bias[h, a].reshape(49)
                    Cfull[49:113, col] = ah_bias[0, h, a, :, 0]
                    Cfull[113:, col] = aw_bias[0, h, a, 0, :]
                else:
                    Cfull[:49, col] = na_bias[h, a].reshape(49)
                    Cfull[49:113, col] = ha_bias[0, h, :, 0, a]
                    Cfull[113:, col] = wa_bias[0, h, 0, :, a]
    c["CA"] = np.ascontiguousarray(Cfull[:128])
    c["CB"] = np.ascontiguousarray(Cfull[128:])

    binsH, binsW = _pool_bins(H, 7), _pool_bins(W, 7)
    s49 = np.zeros((1, AGENT), np.float32)
    for p in range(7):
        for q in range(7):
            lp = binsH[p][1] - binsH[p][0]
            lq = binsW[q][1] - binsW[q][0]
            s49[0, p * 7 + q] = 1.0 / (lp * lq)
    c["S49"] = s49

    w9 = Wdwc[:, 0, :, :].reshape(DIM, 9)
    diag = np.zeros((18, 128, 128), np.float32)
    for t in range(9):
        for ct_ in range(2):
            np.fill_diagonal(diag[t * 2 + ct_], w9[ct_ * 128:(ct_ + 1) * 128, t])
    c["DIAG"] = diag.reshape(18 * 128, 128)
    c["W9"] = np.ascontiguousarray(
        w9.reshape(2, 128, 9).transpose(1, 0, 2))      # [128, 2, 9]

    c["bproj_eff"] = (bproj + Wproj @ bdwc).astype(np.float32).reshape(1, 256)
    c["onesN"] = np.ones((1, 512), np.float32)
    c["zeroR"] = np.zeros((1, 128), np.float32)
    c["ident1"] = np.eye(1, dtype=np.float32)
    c["ident_bf"] = np.eye(128, dtype=np.float32)
    c["ones1"] = np.ones((128, 1), np.float32)
    return c


CONST_SPECS = [
    ("WqT", F16, [128, 2, DIM]),
    ("WkT", F16, [128, 2, DIM]),
    ("WvT", F16, [128, 2, DIM]),
    ("WprojT", F16, [128, 2, DIM]),
    ("PhiA", F16, [128, N]),
    ("PhiB", F16, [49, N]),
    ("CA", F16, [128, 2 * HAP]),
    ("CB", F16, [49, 2 * HAP]),
    ("S49", F32, [1, AGENT]),
    ("W9", F32, [128, 2, 9]),
    ("bproj_eff", F16, [1, DIM]),
    ("onesN", F16, [1, 512]),
    ("zeroR", F16, [1, 128]),
    ("ident1", F32, [1, 1]),
    ("ident_bf", F16, [128, 128]),
    ("ones1", F16, [128, 1]),
]
DIAG_SPEC = ("DIAG", F16, [18 * 128, 128])   # DRAM-resident, streamed


def split_multiwaits(nc, max_waits=1):
    """Walrus codegen has one sync-wait slot per instruction; split extras
    into standalone EventSemaphore waits on the same engine."""
    n_split = 0
    for f in nc.m.functions:
        for bb in f.blocks:
            new_insts = []
            changed = False
            for inst in bb.instructions:
                si = inst.sync_info
                if (si is not None and si.on_wait is not None
                        and len(si.on_wait) > max_waits and inst.is_executable()):
                    waits = list(si.on_wait)
                    extra, keep = waits[:-max_waits], waits[-max_waits:]
                    for w in extra:
                        ev = mybir.InstEventSemaphore(
                            name=f"{inst.name}-ws{n_split}",
                            engine=inst.engine, ins=[], outs=[],
                            sync_info=mybir.SyncInfo(on_wait=[w], on_update=[]),
                        )
                        new_insts.append(ev)
                        n_split += 1
                    inst.sync_info = mybir.SyncInfo(
                        on_wait=keep, on_update=list(si.on_update))
                    changed = True
                new_insts.append(inst)
            if changed:
                bb.instructions = new_insts
    return n_split


def build_nc():
    nc = bass.Bass()
    x_in = nc.dram_tensor("x", [SPC, DIM, N], F16, kind="ExternalInput")
    out_d = nc.dram_tensor("out", [SPC, DIM, N], F32, kind="ExternalOutput")
    cst = {name: nc.dram_tensor(name, shape, dt, kind="ExternalInput")
           for name, dt, shape in CONST_SPECS}
    name, dt, shape = DIAG_SPEC
    cst[name] = nc.dram_tensor(name, shape, dt, kind="ExternalInput")

    with tile.TileContext(nc) as tc, ExitStack() as ctx:
        kernel_body(ctx, tc, nc, x_in, out_d, cst)
    split_multiwaits(nc)
    return nc


def kernel_body(ctx, tc, nc, x_in, out_d, cst):
    const = ctx.enter_context(tc.tile_pool(name="const", bufs=1))
    big = ctx.enter_context(tc.tile_pool(name="big", bufs=1))
    work = ctx.enter_context(tc.tile_pool(name="work", bufs=3))
    small = ctx.enter_context(tc.tile_pool(name="small", bufs=2))
    outp = ctx.enter_context(tc.tile_pool(name="outp", bufs=2))
    # PSUM budget (8 banks): psA ps_main x3 + ps_dw x1, psB x2, psAcc x2
    psA = ctx.enter_context(tc.tile_pool(name="psA", bufs=3, space="PSUM"))
    psB = ctx.enter_context(tc.tile_pool(name="psB", bufs=2, space="PSUM"))
    psAcc = ctx.enter_context(tc.tile_pool(name="psAcc", bufs=1, space="PSUM"))

    ct = {}
    for name, dt, shape in CONST_SPECS:
        t = const.tile(shape, dt, name=f"c_{name}")
        nc.sync.dma_start(out=t, in_=cst[name].ap())
        ct[name] = t
    s49rep = const.tile([128, AGENT], F32, name="s49rep")
    nc.sync.dma_start(
        out=s49rep,
        in_=bass.AP(tensor=cst["S49"], offset=0, ap=[[0, 128], [1, AGENT]]))
    ct["s49rep"] = s49rep
    ct["DIAG_dram"] = cst["DIAG"]

    for s in range(SPC):
        sample(nc, ct, s, x_in, out_d, big, work, small, outp, psA, psB, psAcc)


def sample(nc, ct, s, x_in, out_d, big, work, small, outp, psA, psB, psAcc):
    F = 512
    NCH = N // F

    # ---- X halves (fp16) ------------------------------------------------
    xh = []
    for hf in range(2):
        t = big.tile([128, N], F16, name=f"x{hf}", tag=f"xh{hf}")
        nc.sync.dma_start(out=t, in_=x_in.ap()[s, hf * 128:(hf + 1) * 128, :])
        xh.append(t)

    # ---- QKV projections ------------------------------------------------
    QT, KT, VT = [], [], []
    for hf in range(2):
        QT.append(big.tile([128, N], F16, name=f"qt{hf}", tag=f"qt{hf}"))
        KT.append(big.tile([128, N], F16, name=f"kt{hf}", tag=f"kt{hf}"))
        VT.append(big.tile([128, N], F16, name=f"v{hf}", tag=f"v{hf}"))

    for wname, dest in (("WvT", "v"), ("WkT", "k"), ("WqT", "q")):
        wt = ct[wname]
        for mt in range(2):
            for chn in range(NCH):
                ps = psA.tile([128, F], F32, name="ps_proj", tag="ps_main")
                for kt_ in range(2):
                    nc.tensor.matmul(
                        ps,
                        lhsT=wt[:, kt_, mt * 128:(mt + 1) * 128],
                        rhs=xh[kt_][:, chn * F:(chn + 1) * F],
                        start=(kt_ == 0), stop=(kt_ == 1))
                if dest == "q":
                    nc.vector.tensor_copy(
                        out=QT[mt][:, chn * F:(chn + 1) * F], in_=ps)
                elif dest == "k":
                    nc.vector.tensor_copy(
                        out=KT[mt][:, chn * F:(chn + 1) * F], in_=ps)
                else:
                    nc.scalar.copy(out=VT[mt][:, chn * F:(chn + 1) * F], in_=ps)

    # ---- agent tokens: pool X -> project -> 64-padded blockdiag ---------
    binsH, binsW = _pool_bins(H, 7), _pool_bins(W, 7)
    XpH = []
    for hf in range(2):
        x3 = xh[hf].rearrange("p (y x) -> p y x", y=H)
        qx = small.tile([128, H, 7], F32, name="qx", tag="qx")
        for q, (s0, e0) in enumerate(binsW):
            nc.vector.tensor_reduce(
                out=qx[:, :, q:q + 1], in_=x3[:, :, s0:e0],
                axis=AX.X, op=mybir.AluOpType.add)
        xp = small.tile([128, 7, 7], F32, name="xp", tag="xp")
        qxf = qx.rearrange("p y q -> p (y q)")
        for p, (s0, e0) in enumerate(binsH):
            seg = bass.AP(tensor=qxf.tensor, offset=qxf.offset + s0 * 7,
                          ap=[qxf.ap[0], [1, 7], [7, e0 - s0]])
            nc.vector.tensor_reduce(
                out=xp[:, p, :], in_=seg, axis=AX.X, op=mybir.AluOpType.add)
        xpb = small.tile([128, AGENT], F16, name="xpb", tag="xpb")
        nc.vector.tensor_mul(
            out=xpb, in0=xp.rearrange("p a b -> p (a b)"), in1=ct["s49rep"])
        XpH.append(xpb)

    agentT = []
    for mt in range(2):
        ps = psB.tile([128, AGENT], F32, name="ps_ag", tag="ps_aux")
        for kt_ in range(2):
            nc.tensor.matmul(
                ps,
                lhsT=ct["WqT"][:, kt_, mt * 128:(mt + 1) * 128],
                rhs=XpH[kt_], start=(kt_ == 0), stop=(kt_ == 1))
        at = small.tile([128, AGENT], F16, name=f"at{mt}", tag=f"at{mt}")
        nc.scalar.activation(out=at, in_=ps, func=AF.Copy, scale=SCALE)
        agentT.append(at)

    bd = []
    for hf in range(2):
        b = small.tile([128, 4 * AGP], F16, name=f"bd{hf}", tag=f"bd{hf}")
        nc.vector.memset(b, 0.0)
        for hl in range(4):
            nc.vector.tensor_copy(
                out=b[hl * 32:(hl + 1) * 32, hl * AGP:hl * AGP + AGENT],
                in_=agentT[hf][hl * 32:(hl + 1) * 32, :])
        bd.append(b)

    # ---- dwc: diagonal matmuls, x-wrap columns fixed on DVE ------------
    DWall = big.tile([128, 2, N], F16, name="dwall", tag="xh1")
    TAPS = [(0, 0)] + [(dy, dx) for dy in (-1, 0, 1) for dx in (-1, 0, 1)
                       if (dy, dx) != (0, 0)]
    for cti in range(2):
        dgs = work.tile([128, 9, 128], F16, name="dgs", tag="dgs")
        nc.sync.dma_start(
            out=dgs,
            in_=bass.AP(tensor=ct["DIAG_dram"], offset=cti * 128 * 128,
                        ap=[[128, 128], [2 * 128 * 128, 9], [1, 128]]))
        v = VT[cti]
        for chn in range(NCH):
            ps = psA.tile([128, F], F32, name="ps_dw", tag="ps_dw", bufs=1)
            lo = chn * F
            for k, (dy, dx) in enumerate(TAPS):
                t9 = (dy + 1) * 3 + (dx + 1)
                d = dy * W + dx
                a = max(0, -(lo + d))
                b_ = max(0, (lo + F + d) - N)
                nc.tensor.matmul(
                    ps[:, a:F - b_], lhsT=dgs[:, t9, :],
                    rhs=v[:, lo + d + a:lo + F + d - b_],
                    start=(k == 0), stop=(k == 8), skip_group_check=True)
            nc.scalar.copy(out=DWall[:, cti, lo:lo + F], in_=ps)
        # x-boundary columns: recompute exactly with strided DVE ops
        dwp = DWall[:, cti, :]
        for xb, dxs in ((0, (0, 1)), (W - 1, (-1, 0))):
            first = True
            for dy in (0, -1, 1):     # dy=0 first: full row range overwrite
                for dx in dxs:
                    t9 = (dy + 1) * 3 + (dx + 1)
                    rs, re = max(0, -dy), H - max(0, dy)
                    nr = re - rs
                    o_ap = bass.AP(tensor=dwp.tensor,
                                   offset=dwp.offset + rs * W + xb,
                                   ap=[dwp.ap[0], [W, nr]])
                    v_ap = bass.AP(tensor=v.tensor,
                                   offset=v.offset + (rs + dy) * W + xb + dx,
                                   ap=[v.ap[0], [W, nr]])
                    wcol = ct["W9"][:, cti, t9:t9 + 1]
                    if first:
                        nc.vector.tensor_scalar_mul(
                            out=o_ap, in0=v_ap, scalar1=wcol)
                        first = False
                    else:
                        nc.vector.scalar_tensor_tensor(
                            out=o_ap, in0=v_ap, scalar=wcol, in1=o_ap,
                            op0=mybir.AluOpType.mult, op1=mybir.AluOpType.add)

    # ---- stage 1: agent -> kv attention --------------------------------
    ps_cs = psAcc.tile([1, HAP], F32, name="ps_cs", tag="ps_cs")
    ps_av8 = psAcc.tile([128, 4 * AGP], F32, name="ps_av8", tag="ps_av8")
    nc.tensor.matmul(ps_av8[:, :128], lhsT=ct["zeroR"],
                     rhs=ct["onesN"][:, :128],
                     start=True, stop=False, skip_group_check=True)
    nc.tensor.matmul(ps_av8[:, 128:], lhsT=ct["zeroR"],
                     rhs=ct["onesN"][:, :128],
                     start=True, stop=False, skip_group_check=True)
    for t in range(NT):
        vtok = work.tile([128, DIM], F16, name="vtok", tag="vtok")
        for hf in range(2):
            pst = psB.tile([128, 128], F16, name="ps_vt", tag="ps_aux")
            nc.tensor.transpose(
                pst, in_=VT[hf][:, t * 128:(t + 1) * 128],
                identity=ct["ident_bf"])
            nc.vector.tensor_copy(out=vtok[:, hf * 128:(hf + 1) * 128],
                                  in_=pst)

        ps = psA.tile([128, HAP], F32, name="ps_s1", tag="ps_main")
        nc.tensor.matmul(
            ps, lhsT=ct["PhiA"][:, t * 128:(t + 1) * 128],
            rhs=ct["CA"][:, :HAP], start=True, stop=False,
            skip_group_check=True)
        nc.tensor.matmul(
            ps, lhsT=ct["PhiB"][:, t * 128:(t + 1) * 128],
            rhs=ct["CB"][:, :HAP], start=False, stop=False,
            skip_group_check=True)
        for hf in range(2):
            nc.tensor.matmul(
                ps[:, hf * 4 * AGP:(hf + 1) * 4 * AGP],
                lhsT=KT[hf][:, t * 128:(t + 1) * 128],
                rhs=bd[hf], start=False, stop=True,
                skip_group_check=True)
        e1 = work.tile([128, HAP], F16, name="e1", tag="e")
        nc.scalar.activation(out=e1, in_=ps, func=AF.Exp)
        nc.tensor.matmul(ps_cs, lhsT=ct["ones1"], rhs=e1,
                         start=(t == 0), stop=(t == NT - 1),
                         skip_group_check=True)
        for p in range(NPAIR):
            nc.tensor.matmul(
                ps_av8[:, p * AGP:(p + 1) * AGP],
                lhsT=e1[:, p * 2 * AGP:(p + 1) * 2 * AGP],
                rhs=vtok[:, p * AGP:(p + 1) * AGP],
                start=False, stop=(t == NT - 1),
                skip_group_check=True)

    # normalize agent_v -> AVbd (64-padded blockdiag) -> WpAVT
    cs_sb = small.tile([1, HAP], F32, name="cs_sb", tag="cs_sb")
    nc.scalar.copy(out=cs_sb, in_=ps_cs)
    AVbd = []
    for hf in range(2):
        av = small.tile([128, 4 * AGP], F16, name=f"avbd{hf}", tag=f"avbd{hf}")
        nc.vector.memset(av, 0.0)
        AVbd.append(av)
    for h in range(HEADS):
        p = h // 2
        rb = 64 * (h % 2)
        cb = p * AGP + (h % 2) * HD
        pst = psB.tile([AGENT, 1], F32, name="ps_csT", tag="ps_aux")
        nc.tensor.transpose(
            pst, in_=cs_sb[:, h * AGP:h * AGP + AGENT], identity=ct["ident1"])
        rcp = small.tile([AGENT, 1], F32, name="rcp", tag="rcp")
        nc.vector.reciprocal(out=rcp, in_=pst)
        avn = small.tile([AGENT, HD], F16, name="avn", tag="avn")
        nc.vector.tensor_scalar_mul(
            out=avn, in0=ps_av8[rb:rb + AGENT, cb:cb + HD], scalar1=rcp)
        pst2 = psB.tile([HD, AGENT], F16, name="ps_avT", tag="ps_aux")
        nc.tensor.transpose(
            pst2, in_=avn, identity=ct["ident_bf"][:AGENT, :AGENT])
        hf, hl = h // 4, h % 4
        nc.scalar.copy(
            out=AVbd[hf][hl * HD:(hl + 1) * HD,
                         hl * AGP:hl * AGP + AGENT],
            in_=pst2)

    WpAVT = []
    for p in range(NPAIR):
        hf = p // 2
        ps = psB.tile([128, DIM], F32, name="ps_wpav", tag="ps_aux")
        nc.tensor.matmul(
            ps, lhsT=AVbd[hf][:, (p % 2) * 2 * AGP:((p % 2) + 1) * 2 * AGP],
            rhs=ct["WprojT"][:, hf, :], start=True, stop=True)
        w = small.tile([128, DIM], F16, name=f"wpav{p}", tag=f"wpav{p}")
        nc.scalar.copy(out=w, in_=ps)
        WpAVT.append(w)

    # ---- stage 2: query -> agent attention -----------------------------
    A2Tall = big.tile([128, NPAIR, N], F16, name="a2tall", tag="kt0")
    for t in range(NT):
        ps = psA.tile([128, HAP], F32, name="ps_s2", tag="ps_main")
        nc.tensor.matmul(
            ps, lhsT=ct["PhiA"][:, t * 128:(t + 1) * 128],
            rhs=ct["CA"][:, HAP:], start=True, stop=False,
            skip_group_check=True)
        nc.tensor.matmul(
            ps, lhsT=ct["PhiB"][:, t * 128:(t + 1) * 128],
            rhs=ct["CB"][:, HAP:], start=False, stop=False,
            skip_group_check=True)
        for hf in range(2):
            nc.tensor.matmul(
                ps[:, hf * 4 * AGP:(hf + 1) * 4 * AGP],
                lhsT=QT[hf][:, t * 128:(t + 1) * 128],
                rhs=bd[hf], start=False, stop=True,
                skip_group_check=True)
        e2 = work.tile([128, HAP], F16, name="e2", tag="e")
        nc.scalar.activation(out=e2, in_=ps, func=AF.Exp)
        s2 = work.tile([128, HEADS], F32, name="s2", tag="s2")
        nc.vector.tensor_reduce(
            out=s2, in_=e2.rearrange("p (h a) -> p h a", h=HEADS),
            axis=AX.X, op=mybir.AluOpType.add)
        r2 = work.tile([128, HEADS], F32, name="r2", tag="r2")
        nc.vector.reciprocal(out=r2, in_=s2)
        a2 = work.tile([128, HAP], F16, name="a2", tag="a2")
        r2v = bass.AP(tensor=r2.tensor, offset=r2.offset,
                      ap=[r2.ap[0], [1, HEADS], [0, AGP]])
        nc.vector.tensor_mul(
            out=a2.rearrange("p (h a) -> p h a", h=HEADS),
            in0=e2.rearrange("p (h a) -> p h a", h=HEADS), in1=r2v)
        for p in range(NPAIR):
            pst = psB.tile([128, 128], F16, name="ps_a2t", tag="ps_aux")
            nc.tensor.transpose(
                pst, in_=a2[:, p * 2 * AGP:(p + 1) * 2 * AGP],
                identity=ct["ident_bf"])
            if p % 2 == 0:
                nc.vector.tensor_copy(
                    out=A2Tall[:, p, t * 128:(t + 1) * 128], in_=pst)
            else:
                nc.scalar.copy(
                    out=A2Tall[:, p, t * 128:(t + 1) * 128], in_=pst)

    # ---- fused output ---------------------------------------------------
    for ot in range(2):
        for chn in range(NCH):
            ps = psA.tile([128, F], F32, name="ps_o", tag="ps_main")
            for p in range(NPAIR):
                nc.tensor.matmul(
                    ps, lhsT=WpAVT[p][:, ot * 128:(ot + 1) * 128],
                    rhs=A2Tall[:, p, chn * F:(chn + 1) * F],
                    start=(p == 0), stop=False)
            for kt_ in range(2):
                nc.tensor.matmul(
                    ps,
                    lhsT=ct["WprojT"][:, kt_, ot * 128:(ot + 1) * 128],
                    rhs=DWall[:, kt_, chn * F:(chn + 1) * F],
                    start=False, stop=False)
            nc.tensor.matmul(
                ps, lhsT=ct["bproj_eff"][:, ot * 128:(ot + 1) * 128],
                rhs=ct["onesN"], start=False, stop=True)
            o = outp.tile([128, F], F32, name="o_st", tag="o_st")
            nc.scalar.copy(out=o, in_=ps)
            nc.gpsimd.dma_start(
                out=out_d.ap()[s, ot * 128:(ot + 1) * 128,
                               chn * F:(chn + 1) * F],
                in_=o)


def kernel(**inputs):
    x = np.asarray(inputs["x"], np.float32)
    host = build_host_constants(
        np.asarray(inputs["Wq"], np.float32),
        np.asarray(inputs["Wkv"], np.float32),
        np.asarray(inputs["Wproj"], np.float32),
        np.asarray(inputs["bproj"], np.float32),
        np.asarray(inputs["Wdwc"], np.float32),
        np.asarray(inputs["bdwc"], np.float32),
        np.asarray(inputs["an_bias"], np.float32),
        np.asarray(inputs["na_bias"], np.float32),
        np.asarray(inputs["ah_bias"], np.float32),
        np.asarray(inputs["aw_bias"], np.float32),
        np.asarray(inputs["ha_bias"], np.float32),
        np.asarray(inputs["wa_bias"], np.float32),
    )
    nc = build_nc()

    specs = CONST_SPECS + [DIAG_SPEC]
    dts = {name: dt for name, dt, _ in specs}

    def cast(name, arr):
        return np.asarray(arr, np.float16 if dts[name] == F16 else np.float32)

    const_map = {name: cast(name, host[name]) for name, _, _ in specs}
    xs = x.reshape(B, DIM, N)
    in_maps = []
    for c in range(N_CORES):
        m = dict(const_map)
        m["x"] = np.asarray(xs[c * SPC:(c + 1) * SPC], np.float16)
        in_maps.append(m)

    res = run_bass_kernel_spmd(nc, in_maps, core_ids=list(range(N_CORES)))
    out = np.concatenate([res.results[c]["out"] for c in range(N_CORES)],
                         axis=0)
    return out.reshape(B, DIM, H, W)
